# revision 4
# baseline (speedup 1.0000x reference)
"""GNN message-passing layer (gnn_message_passing) on 8 Trainium2 cores.

Data-parallel over nodes: each core gets N/8 nodes (+ padding) with inputs
pre-transposed on host to feature-major [128, cols] layout so the device
streams them straight into matmuls without on-chip transposes.

Math (per node n, neighbors j=0..15):
  s[n,j] = x_n[n,j]@ (W@w2) + x_s[n] @ (W@w1) + (b@(w1+w2) + b_att)
  att    = softmax_j(sigmoid(s))
  Z[n]   = sum_j att[n,j] * x_n[n,j]
  out    = relu((0.5*x_s + 0.5*Z) @ W + b)
"""

import sys

sys.path.insert(0, "/opt/trn_rl_repo")

import numpy as np

N, NEIGH, D = 50000, 16, 128
N_CORES = 8
NPC = N // N_CORES          # 6250 nodes per core
BN = 128                    # nodes per block
NB = (NPC + BN - 1) // BN   # 49 blocks
NPAD = NB * BN              # 6272 padded nodes per core
BC = BN * NEIGH             # 2048 neighbor columns per block
CH = 4                      # broadcast/multiply chunks per block
CHN = BN // CH              # 32 nodes per chunk
CHC = CHN * NEIGH           # 512 cols per chunk

_compiled = {}


def _build():
    if "nc" in _compiled:
        return _compiled["nc"]

    import concourse.bacc as bacc
    import concourse.mybir as mybir
    import concourse.tile as tile

    f32 = mybir.dt.float32
    AX = mybir.AxisListType
    ALU = mybir.AluOpType
    ACTF = mybir.ActivationFunctionType

    nc = bacc.Bacc("TRN2", target_bir_lowering=False, debug=False,
                   num_devices=N_CORES)

    nxt = nc.dram_tensor("nxt", [D, NPAD * NEIGH], f32, kind="ExternalInput")
    sxt = nc.dram_tensor("sxt", [D, NPAD], f32, kind="ExternalInput")
    v2c = nc.dram_tensor("v2c", [D, 1], f32, kind="ExternalInput")
    v1r = nc.dram_tensor("v1r", [D, NEIGH], f32, kind="ExternalInput")
    whalf = nc.dram_tensor("whalf", [D, D], f32, kind="ExternalInput")
    onesr = nc.dram_tensor("onesr", [1, D], f32, kind="ExternalInput")
    brow = nc.dram_tensor("brow", [1, D], f32, kind="ExternalInput")
    cconst = nc.dram_tensor("cconst", [D, 1], f32, kind="ExternalInput")
    outd = nc.dram_tensor("out", [NPAD, D], f32, kind="ExternalOutput")
    attd = nc.dram_tensor("att", [NPAD, NEIGH], f32, kind="ExternalOutput")

    with tile.TileContext(nc) as tc:
        with (
            tc.tile_pool(name="consts", bufs=1) as consts,
            tc.tile_pool(name="xt", bufs=3) as xt_pool,
            tc.tile_pool(name="sx", bufs=3) as sx_pool,
            tc.tile_pool(name="tmp", bufs=2) as tmp_pool,
            tc.tile_pool(name="small", bufs=4) as small,
            tc.tile_pool(name="rows", bufs=3) as rows,
            tc.tile_pool(name="outs", bufs=3) as outs_pool,
            tc.tile_pool(name="score_ps", bufs=2, space="PSUM") as score_pool,
            tc.tile_pool(name="ab_ps", bufs=3, space="PSUM") as ab_pool,
            tc.tile_pool(name="out_ps", bufs=2, space="PSUM") as outp_pool,
        ):
            v2c_sb = consts.tile([D, 1], f32)
            nc.sync.dma_start(out=v2c_sb, in_=v2c[:, :])
            v1r_sb = consts.tile([D, NEIGH], f32)
            nc.sync.dma_start(out=v1r_sb, in_=v1r[:, :])
            whalf_sb = consts.tile([D, D], f32)
            nc.sync.dma_start(out=whalf_sb, in_=whalf[:, :])
            onesr_sb = consts.tile([1, D], f32)
            nc.sync.dma_start(out=onesr_sb, in_=onesr[:, :])
            brow_sb = consts.tile([1, D], f32)
            nc.sync.dma_start(out=brow_sb, in_=brow[:, :])
            cc_sb = consts.tile([D, 1], f32)
            nc.sync.dma_start(out=cc_sb, in_=cconst[:, :])

            for blk in range(NB):
                xt = xt_pool.tile([D, BC], f32)
                nc.sync.dma_start(out=xt, in_=nxt[:, blk * BC:(blk + 1) * BC])
                sx = sx_pool.tile([D, BN], f32)
                nc.sync.dma_start(out=sx, in_=sxt[:, blk * BN:(blk + 1) * BN])

                # score[n, j] = x_n[n,j]@v2 + x_s[n]@v1, in PSUM [128, 16]
                score_ps = score_pool.tile([BN, NEIGH], f32)
                for j in range(NEIGH):
                    nc.tensor.matmul(
                        score_ps[:, j:j + 1],
                        lhsT=xt[:, j * BN:(j + 1) * BN],
                        rhs=v2c_sb,
                        start=(j == 0), stop=False, skip_group_check=True,
                    )
                nc.tensor.matmul(
                    score_ps, lhsT=sx, rhs=v1r_sb,
                    start=False, stop=True, skip_group_check=True,
                )

                # att = softmax_j(sigmoid(score + C))
                sig = small.tile([BN, NEIGH], f32)
                nc.scalar.activation(sig, score_ps, ACTF.Sigmoid, bias=cc_sb)
                e2 = small.tile([BN, NEIGH], f32)
                nc.scalar.activation(e2, sig, ACTF.Exp)
                ssum = small.tile([BN, 1], f32)
                nc.vector.tensor_reduce(ssum, e2, axis=AX.X, op=ALU.add)
                rec = small.tile([BN, 1], f32)
                nc.vector.reciprocal(rec, ssum)
                att2 = small.tile([BN, NEIGH], f32)
                nc.vector.tensor_scalar_mul(att2, e2, rec)
                nc.sync.dma_start(out=attd[blk * BN:(blk + 1) * BN, :], in_=att2)

                # flatten att [128,16] -> row [1, 2048] (node-major)
                arow = rows.tile([1, BC], f32)
                nc.sync.dma_start(out=arow, in_=att2)

                # Z[d, n] = sum_j xt[d, j*128+n] * att[n, j]
                tmp = tmp_pool.tile([D, BN, NEIGH], f32)
                xtv = xt.rearrange("p (j n) -> p n j", j=NEIGH)
                for c in range(CH):
                    ab = ab_pool.tile([D, CHC], f32)
                    nc.tensor.matmul(
                        ab, lhsT=onesr_sb, rhs=arow[:, c * CHC:(c + 1) * CHC],
                        start=True, stop=True,
                    )
                    abv = ab.rearrange("p (n j) -> p n j", j=NEIGH)
                    nc.vector.tensor_mul(
                        tmp[:, c * CHN:(c + 1) * CHN, :],
                        xtv[:, c * CHN:(c + 1) * CHN, :],
                        abv,
                    )
                zt = small.tile([D, BN], f32)
                nc.vector.tensor_reduce(zt, tmp, axis=AX.X, op=ALU.add)

                # m = x_s + Z  (0.5 factor folded into whalf)
                mt = small.tile([D, BN], f32)
                nc.vector.tensor_add(mt, sx, zt)

                # out[n, :] = relu(m.T @ (0.5 W) + b)
                outp = outp_pool.tile([BN, D], f32)
                nc.tensor.matmul(outp, lhsT=onesr_sb, rhs=brow_sb,
                                 start=True, stop=False, skip_group_check=True)
                nc.tensor.matmul(outp, lhsT=mt, rhs=whalf_sb,
                                 start=False, stop=True, skip_group_check=True)
                outs = outs_pool.tile([BN, D], f32)
                nc.scalar.activation(outs, outp, ACTF.Relu)
                nc.sync.dma_start(out=outd[blk * BN:(blk + 1) * BN, :], in_=outs)

    nc.compile()
    _compiled["nc"] = nc
    return nc


def prepare_in_maps(self_emb, neigh_emb, W, b, w_att, b_att):
    self_emb = np.asarray(self_emb, dtype=np.float32)
    neigh_emb = np.asarray(neigh_emb, dtype=np.float32)
    W = np.asarray(W, dtype=np.float32)
    b = np.asarray(b, dtype=np.float32)
    w_att = np.asarray(w_att, dtype=np.float32)
    b_att = np.asarray(b_att, dtype=np.float32)

    w1, w2 = w_att[:D], w_att[D:]
    v1 = (W @ w1).astype(np.float32)
    v2 = (W @ w2).astype(np.float32)
    cscal = np.float32(b @ (w1 + w2) + b_att[0])

    v2c = np.ascontiguousarray(v2[:, None])
    v1r = np.ascontiguousarray(np.tile(v1[:, None], (1, NEIGH)))
    whalf = np.ascontiguousarray(0.5 * W)
    onesr = np.ones((1, D), dtype=np.float32)
    brow = np.ascontiguousarray(b[None, :])
    cconst = np.full((D, 1), cscal, dtype=np.float32)

    in_maps = []
    for c in range(N_CORES):
        s = self_emb[c * NPC:(c + 1) * NPC]
        spad = np.zeros((NPAD, D), dtype=np.float32)
        spad[:NPC] = s
        sxt = np.ascontiguousarray(spad.T)

        nshard = neigh_emb[c * NPC * NEIGH:(c + 1) * NPC * NEIGH]
        npad = np.zeros((NPAD * NEIGH, D), dtype=np.float32)
        npad[:NPC * NEIGH] = nshard
        # [NPAD*NEIGH, D] -> (blk, n, j, d) -> (d, blk, j, n) -> [D, NPAD*NEIGH]
        nxt = np.ascontiguousarray(
            npad.reshape(NB, BN, NEIGH, D).transpose(3, 0, 2, 1).reshape(D, -1)
        )
        in_maps.append({
            "nxt": nxt, "sxt": sxt, "v2c": v2c, "v1r": v1r,
            "whalf": whalf, "onesr": onesr, "brow": brow, "cconst": cconst,
        })
    return in_maps


def postprocess(results):
    out_full = np.empty((N, D), dtype=np.float32)
    att_full = np.empty((N, NEIGH), dtype=np.float32)
    for c in range(N_CORES):
        out_full[c * NPC:(c + 1) * NPC] = results[c]["out"][:NPC]
        att_full[c * NPC:(c + 1) * NPC] = results[c]["att"][:NPC]
    return out_full, att_full


def kernel(self_emb, neigh_emb, W, b, w_att, b_att):
    nc = _build()
    from concourse.bass_utils import run_bass_kernel_spmd

    in_maps = prepare_in_maps(self_emb, neigh_emb, W, b, w_att, b_att)
    res = run_bass_kernel_spmd(nc, in_maps, list(range(N_CORES)))
    return postprocess(res.results)
